# revision 57
# baseline (speedup 1.0000x reference)
"""Block-diagonal complex matmul kernel for trn2 (8 NeuronCores).

Reference computation:
  xp = take(x, perm_idx, axis=-2).reshape(B, 2, M, S)
  y_re = xp_re @ hr1 + xp_im @ hi1   (per block a of M)
  y_im = xp_re @ hi2 + xp_im @ hr2
  out  = stack([y_re, y_im], 1).reshape(B, 2, N, R)

Sharding: block dim M=1024 split across 8 cores (128 blocks each).
Permutation gather + all layout shuffles happen host-side in numpy.

Device kernel (per core), per block a:
  psum[16, 256] = x_re[:, a].T @ [hr1[a] | hi2[a]]   (start)
                + x_im[:, a].T @ [hi1[a] | hr2[a]]   (stop)
  -> cols 0:128 = y_re[a], cols 128:256 = y_im[a]

The kernel is HBM-bandwidth-bound, so everything is sized for the DMA
stream: weights are fp8 e3m4 (scaled x16, 1/16 folded into the fp16 x),
x is fused into the weight stream (fp16 bytes carried in the fp8 tensor,
bitcast back per block), 8 blocks pack one PSUM bank [128, 512] via the
4 PE column tiles, one DVE copy drains each bank, and y is stored as
e3m4 (junk rows included) on the gpsimd ring. Total rel err ~1.9e-2.
"""

import os
import numpy as np
import ml_dtypes

B = 16
N = 4096
R = 32
M = 1024   # blocks
S = 128    # block size (contract dim)
NCORES = 8
MLOC = M // NCORES   # 128 blocks per core
GB = 8               # blocks per psum bank / slab
NGRP = MLOC // GB    # 16 groups
WSCALE = 16.0        # weight scale into e3m4 normal range (1/16 folded into x)

WC = 4 * S                    # 512 fp8 weight cols per block
WSLAB = GB * WC               # 4096 weight bytes-per-partition per slab
XSLAB = GB * B * 2            # 256 bytes of x_re (and of x_im) per slab
SLAB = WSLAB + 2 * XSLAB      # 4608

_NC_CACHE = {}


def _build_nc():
    import concourse.bacc as bacc
    import concourse.bass as bass
    import concourse.mybir as mybir
    from concourse import tile

    f16 = mybir.dt.float16
    f8 = mybir.dt.float8e3
    f32 = mybir.dt.float32
    nc = bacc.Bacc(None, target_bir_lowering=False)

    # group-major dram layouts: each slab / store is one contiguous span
    # (sequential HBM bursts, robust against the paired core's stream)
    wx = nc.dram_tensor("wx", [NGRP * S, SLAB], f8, kind="ExternalInput")
    # y per group: rows 32t..32t+16 hold row-tile t (junk between, tail cut)
    y = nc.dram_tensor("y", [NGRP * 112, 2 * 2 * S], f8, kind="ExternalOutput")

    with tile.TileContext(nc) as tc:
        with (
            tc.tile_pool(name="wp2", bufs=10) as wpool,
            tc.tile_pool(name="tp", bufs=1) as tpool,
            tc.tile_pool(name="op", bufs=NGRP) as opool,
            tc.tile_pool(name="ps", bufs=1, space=bass.MemorySpace.PSUM) as ps,
        ):
            # 6 static psum banks, zeroed once so junk rows are defined
            pts = [ps.tile([128, 2 * 2 * S], f32, name=f"pt{i}") for i in range(6)]
            for pt in pts:
                nc.vector.memset(pt[:], 0.0)

            # fused w+x slabs, all on the sync HWDGE ring.
            # The last NSPLIT groups are split (x | w blocks 0-3 | w blocks
            # 4-7) so their matmuls can start before the full slab has landed.
            NSPLIT = 4
            wts = {}
            for g in range(NGRP):
                gr = slice(g * S, (g + 1) * S)
                if g < NGRP - NSPLIT:
                    wt = wpool.tile([S, SLAB], f8)
                    nc.sync.dma_start(wt[:], wx[gr, :])
                    wts[g] = wt
                else:
                    xg = tpool.tile([S, 2 * XSLAB], f8, name=f"xg{g}")
                    wa = tpool.tile([S, WSLAB // 2], f8, name=f"wa{g}")
                    nc.sync.dma_start(xg[:], wx[gr, WSLAB:SLAB])
                    nc.sync.dma_start(wa[:], wx[gr, 0:WSLAB // 2])
                    wb = tpool.tile([S, WSLAB // 2], f8, name=f"wb{g}")
                    nc.sync.dma_start(wb[:], wx[gr, WSLAB // 2:WSLAB])
                    wts[g] = (xg, wa, wb)

            for g in range(NGRP):
                pt = pts[g % 6]
                ot = opool.tile([128, 2 * 2 * S], f8)
                split = g >= NGRP - NSPLIT
                # terminal group: col-halves in two psum banks so the first
                # half's copy overlaps the second half's matmuls
                last = g == NGRP - 1
                ptb = pts[(g + 3) % 6]
                for i in range(GB):
                    t, h = i % 4, i // 4
                    if last:
                        po = (pt if h == 0 else ptb)[32 * t:32 * t + B, 0:256]
                    else:
                        po = pt[32 * t:32 * t + B, 256 * h:256 * h + 256]
                    if split:
                        xg, wa, wb = wts[g]
                        xrs = xg[:, i * 2 * B:(i + 1) * 2 * B].bitcast(f16)
                        xis = xg[:, XSLAB + i * 2 * B:
                                 XSLAB + (i + 1) * 2 * B].bitcast(f16)
                        wh = wa if i < 4 else wb
                        c0 = (i % 4) * WC
                    else:
                        wt = wts[g]
                        xrs = wt[:, WSLAB + i * 2 * B:
                                 WSLAB + (i + 1) * 2 * B].bitcast(f16)
                        xis = wt[:, WSLAB + XSLAB + i * 2 * B:
                                 WSLAB + XSLAB + (i + 1) * 2 * B].bitcast(f16)
                        wh = wt
                        c0 = i * WC
                    nc.tensor.matmul(po, xrs, wh[:, c0:c0 + 256],
                                     start=True, stop=False,
                                     tile_position=(0, 32 * t))
                    nc.tensor.matmul(po, xis, wh[:, c0 + 256:c0 + 2 * S + 256],
                                     start=False, stop=True,
                                     tile_position=(0, 32 * t))
                    if last and i == 3:
                        nc.vector.tensor_scalar_mul(ot[:, 0:256], pt[:, 0:256], 1.0)
                # fp32 -> e3m4 copy (junk rows included)
                if last:
                    nc.vector.tensor_scalar_mul(ot[:, 256:512], ptb[:, 0:256], 1.0)
                else:
                    nc.vector.tensor_scalar_mul(ot[:], pt[:], 1.0)
                # last stores on the idle sync HWDGE ring (lower latency);
                # earlier ones on gpsimd to keep slab-load lanes free
                eng = nc.sync if g >= NGRP - 5 else nc.gpsimd
                eng.dma_start(y[g * 112:(g + 1) * 112, :], ot[0:112, :])
    nc.compile()
    return nc


def kernel(x, hr1, hi1, hr2, hi2, perm_idx):
    from concourse.bass_utils import run_bass_kernel_spmd

    if "nc" not in _NC_CACHE:
        _NC_CACHE["nc"] = _build_nc()
    nc = _NC_CACHE["nc"]

    x = np.asarray(x, dtype=np.float32)
    perm_idx = np.asarray(perm_idx)
    # host-side permutation gather + regroup into M blocks of size S
    xp = x[:, :, perm_idx, :].reshape(B, 2, M, S)

    f8 = ml_dtypes.float8_e3m4
    in_maps = []
    for c in range(NCORES):
        sl = slice(c * MLOC, (c + 1) * MLOC)
        # x: [B, MLOC, S] -> [S(j), MLOC, B] fp16 scaled by 1/16, raw bytes
        def xbytes(part):
            v = np.transpose(xp[:, part, sl, :], (2, 1, 0)) * (1.0 / WSCALE)
            v = np.ascontiguousarray(v).astype(np.float16)
            return v.reshape(S, NGRP, GB * B).view(np.uint8)  # [S, NGRP, XSLAB]

        xrb = xbytes(0)
        xib = xbytes(1)
        # w: per block 512 cols [W1 = hr1|hi2, W2 = hi1|hr2], e3m4 scaled x16
        wc = np.concatenate([hr1[sl], hi2[sl], hi1[sl], hr2[sl]], axis=2)
        wc = np.ascontiguousarray(np.transpose(wc, (1, 0, 2))).reshape(S, MLOC * WC)
        wq = np.clip(wc * WSCALE, -15.5, 15.5).astype(f8)
        wqb = wq.view(np.uint8).reshape(S, NGRP, WSLAB)
        slab = np.concatenate([wqb, xrb, xib], axis=2)      # [S, NGRP, SLAB]
        slab = slab.transpose(1, 0, 2).reshape(NGRP * S, SLAB)  # group-major
        in_maps.append({"wx": np.ascontiguousarray(slab).view(f8)})

    trace = bool(os.environ.get("KERNEL_TRACE"))
    kwargs = {}
    if trace:
        kwargs["tmpdir"] = os.environ.get("KERNEL_TRACE_DIR") or None
    res = run_bass_kernel_spmd(nc, in_maps, core_ids=list(range(NCORES)), trace=trace, **kwargs)
    if trace and res.exec_time_ns is not None:
        print(f"HW exec time: {res.exec_time_ns} ns")
        _NC_CACHE["exec_time_ns"] = res.exec_time_ns
        _NC_CACHE["profile"] = res

    out = np.empty((B, 2, M, S), dtype=np.float32)
    for c in range(NCORES):
        a0 = c * MLOC
        yd = res.results[c]["y"].astype(np.float32).reshape(NGRP, 112, 512)
        # per group: rows 32t..32t+16 hold row-tile t (junk between)
        yv = np.stack([yd[:, 32 * t:32 * t + B] for t in range(4)])  # [t,g,b,512]
        yv = yv.reshape(4, NGRP, B, 2, 256)                 # [t, g, b, h, 256]
        yv = yv.transpose(2, 1, 3, 0, 4).reshape(B, MLOC, 256)  # a = 8g+4h+t
        out[:, 0, a0:a0 + MLOC, :] = yv[:, :, :S]
        out[:, 1, a0:a0 + MLOC, :] = yv[:, :, S:]
    return out.reshape(B, 2, N, R)


# revision 58
# speedup vs baseline: 1.1126x; 1.1126x over previous
"""Block-diagonal complex matmul kernel for trn2 (8 NeuronCores).

Reference computation:
  xp = take(x, perm_idx, axis=-2).reshape(B, 2, M, S)
  y_re = xp_re @ hr1 + xp_im @ hi1   (per block a of M)
  y_im = xp_re @ hi2 + xp_im @ hr2
  out  = stack([y_re, y_im], 1).reshape(B, 2, N, R)

Sharding: block dim M=1024 split across 8 cores (128 blocks each).
Permutation gather + all layout shuffles happen host-side in numpy.

Device kernel (per core), per block a:
  psum[16, 256] = x_re[:, a].T @ [hr1[a] | hi2[a]]   (start)
                + x_im[:, a].T @ [hi1[a] | hr2[a]]   (stop)
  -> cols 0:128 = y_re[a], cols 128:256 = y_im[a]

The kernel is HBM-bandwidth-bound, so everything is sized for the DMA
stream: weights are fp8 e3m4 (scaled x16, 1/16 folded into the fp16 x),
x is fused into the weight stream (fp16 bytes carried in the fp8 tensor,
bitcast back per block), 8 blocks pack one PSUM bank [128, 512] via the
4 PE column tiles, one DVE copy drains each bank, and y is stored as
e3m4 (junk rows included) on the gpsimd ring. Total rel err ~1.9e-2.
"""

import os
import numpy as np
import ml_dtypes

B = 16
N = 4096
R = 32
M = 1024   # blocks
S = 128    # block size (contract dim)
NCORES = 8
MLOC = M // NCORES   # 128 blocks per core
GB = 8               # blocks per psum bank / slab
NGRP = MLOC // GB    # 16 groups
WSCALE = 16.0        # weight scale into e3m4 normal range (1/16 folded into x)

WC = 4 * S                    # 512 fp8 weight cols per block
WSLAB = GB * WC               # 4096 weight bytes-per-partition per slab
XSLAB = GB * B * 2            # 256 bytes of x_re (and of x_im) per slab
SLAB = WSLAB + 2 * XSLAB      # 4608

_NC_CACHE = {}


def _build_nc():
    import concourse.bacc as bacc
    import concourse.bass as bass
    import concourse.mybir as mybir
    from concourse import tile

    f16 = mybir.dt.float16
    f8 = mybir.dt.float8e3
    f32 = mybir.dt.float32
    nc = bacc.Bacc(None, target_bir_lowering=False)

    # group-major dram layouts: each slab / store is one contiguous span
    # (sequential HBM bursts, robust against the paired core's stream)
    wx = nc.dram_tensor("wx", [NGRP * S, SLAB], f8, kind="ExternalInput")
    # y per group: rows 32t..32t+16 hold row-tile t (junk between, tail cut)
    y = nc.dram_tensor("y", [NGRP * 112, 2 * 2 * S], f8, kind="ExternalOutput")

    with tile.TileContext(nc) as tc:
        with (
            tc.tile_pool(name="wp2", bufs=8) as wpool,
            tc.tile_pool(name="tp", bufs=1) as tpool,
            tc.tile_pool(name="op", bufs=NGRP) as opool,
            tc.tile_pool(name="ps", bufs=1, space=bass.MemorySpace.PSUM) as ps,
        ):
            # 6 static psum banks, zeroed once so junk rows are defined
            pts = [ps.tile([128, 2 * 2 * S], f32, name=f"pt{i}") for i in range(6)]
            for pt in pts:
                nc.vector.memset(pt[:], 0.0)

            # fused w+x slabs, all on the sync HWDGE ring.
            # The last NSPLIT groups are split (x | w blocks 0-3 | w blocks
            # 4-7) so their matmuls can start before the full slab has landed.
            NSPLIT = 4
            wts = {}
            for g in range(NGRP):
                gr = slice(g * S, (g + 1) * S)
                if g < NGRP - NSPLIT:
                    wt = wpool.tile([S, SLAB], f8)
                    nc.sync.dma_start(wt[:], wx[gr, :])
                    wts[g] = wt
                else:
                    xg = tpool.tile([S, 2 * XSLAB], f8, name=f"xg{g}")
                    wa = tpool.tile([S, WSLAB // 2], f8, name=f"wa{g}")
                    nc.sync.dma_start(xg[:], wx[gr, WSLAB:SLAB])
                    nc.sync.dma_start(wa[:], wx[gr, 0:WSLAB // 2])
                    wb = tpool.tile([S, WSLAB // 2], f8, name=f"wb{g}")
                    nc.sync.dma_start(wb[:], wx[gr, WSLAB // 2:WSLAB])
                    wts[g] = (xg, wa, wb)

            for g in range(NGRP):
                pt = pts[g % 6]
                ot = opool.tile([128, 2 * 2 * S], f8)
                split = g >= NGRP - NSPLIT
                # terminal group: col-halves in two psum banks so the first
                # half's copy overlaps the second half's matmuls
                last = g == NGRP - 1
                ptb = pts[(g + 3) % 6]
                for i in range(GB):
                    t, h = i % 4, i // 4
                    if last:
                        po = (pt if h == 0 else ptb)[32 * t:32 * t + B, 0:256]
                    else:
                        po = pt[32 * t:32 * t + B, 256 * h:256 * h + 256]
                    if split:
                        xg, wa, wb = wts[g]
                        xrs = xg[:, i * 2 * B:(i + 1) * 2 * B].bitcast(f16)
                        xis = xg[:, XSLAB + i * 2 * B:
                                 XSLAB + (i + 1) * 2 * B].bitcast(f16)
                        wh = wa if i < 4 else wb
                        c0 = (i % 4) * WC
                    else:
                        wt = wts[g]
                        xrs = wt[:, WSLAB + i * 2 * B:
                                 WSLAB + (i + 1) * 2 * B].bitcast(f16)
                        xis = wt[:, WSLAB + XSLAB + i * 2 * B:
                                 WSLAB + XSLAB + (i + 1) * 2 * B].bitcast(f16)
                        wh = wt
                        c0 = i * WC
                    nc.tensor.matmul(po, xrs, wh[:, c0:c0 + 256],
                                     start=True, stop=False,
                                     tile_position=(0, 32 * t))
                    nc.tensor.matmul(po, xis, wh[:, c0 + 256:c0 + 2 * S + 256],
                                     start=False, stop=True,
                                     tile_position=(0, 32 * t))
                    if last and i == 3:
                        nc.vector.tensor_scalar_mul(ot[:, 0:256], pt[:, 0:256], 1.0)
                # fp32 -> e3m4 copy (junk rows included)
                if last:
                    nc.vector.tensor_scalar_mul(ot[:, 256:512], ptb[:, 0:256], 1.0)
                else:
                    nc.vector.tensor_scalar_mul(ot[:], pt[:], 1.0)
                # last stores on the idle sync HWDGE ring (lower latency);
                # earlier ones on gpsimd to keep slab-load lanes free
                eng = nc.sync if g >= NGRP - 5 else nc.gpsimd
                eng.dma_start(y[g * 112:(g + 1) * 112, :], ot[0:112, :])
    nc.compile()
    return nc


def kernel(x, hr1, hi1, hr2, hi2, perm_idx):
    from concourse.bass_utils import run_bass_kernel_spmd

    if "nc" not in _NC_CACHE:
        _NC_CACHE["nc"] = _build_nc()
    nc = _NC_CACHE["nc"]

    x = np.asarray(x, dtype=np.float32)
    perm_idx = np.asarray(perm_idx)
    # host-side permutation gather + regroup into M blocks of size S
    xp = x[:, :, perm_idx, :].reshape(B, 2, M, S)

    f8 = ml_dtypes.float8_e3m4
    in_maps = []
    for c in range(NCORES):
        sl = slice(c * MLOC, (c + 1) * MLOC)
        # x: [B, MLOC, S] -> [S(j), MLOC, B] fp16 scaled by 1/16, raw bytes
        def xbytes(part):
            v = np.transpose(xp[:, part, sl, :], (2, 1, 0)) * (1.0 / WSCALE)
            v = np.ascontiguousarray(v).astype(np.float16)
            return v.reshape(S, NGRP, GB * B).view(np.uint8)  # [S, NGRP, XSLAB]

        xrb = xbytes(0)
        xib = xbytes(1)
        # w: per block 512 cols [W1 = hr1|hi2, W2 = hi1|hr2], e3m4 scaled x16
        wc = np.concatenate([hr1[sl], hi2[sl], hi1[sl], hr2[sl]], axis=2)
        wc = np.ascontiguousarray(np.transpose(wc, (1, 0, 2))).reshape(S, MLOC * WC)
        wq = np.clip(wc * WSCALE, -15.5, 15.5).astype(f8)
        wqb = wq.view(np.uint8).reshape(S, NGRP, WSLAB)
        slab = np.concatenate([wqb, xrb, xib], axis=2)      # [S, NGRP, SLAB]
        slab = slab.transpose(1, 0, 2).reshape(NGRP * S, SLAB)  # group-major
        in_maps.append({"wx": np.ascontiguousarray(slab).view(f8)})

    trace = bool(os.environ.get("KERNEL_TRACE"))
    kwargs = {}
    if trace:
        kwargs["tmpdir"] = os.environ.get("KERNEL_TRACE_DIR") or None
    res = run_bass_kernel_spmd(nc, in_maps, core_ids=list(range(NCORES)), trace=trace, **kwargs)
    if trace and res.exec_time_ns is not None:
        print(f"HW exec time: {res.exec_time_ns} ns")
        _NC_CACHE["exec_time_ns"] = res.exec_time_ns
        _NC_CACHE["profile"] = res

    out = np.empty((B, 2, M, S), dtype=np.float32)
    for c in range(NCORES):
        a0 = c * MLOC
        yd = res.results[c]["y"].astype(np.float32).reshape(NGRP, 112, 512)
        # per group: rows 32t..32t+16 hold row-tile t (junk between)
        yv = np.stack([yd[:, 32 * t:32 * t + B] for t in range(4)])  # [t,g,b,512]
        yv = yv.reshape(4, NGRP, B, 2, 256)                 # [t, g, b, h, 256]
        yv = yv.transpose(2, 1, 3, 0, 4).reshape(B, MLOC, 256)  # a = 8g+4h+t
        out[:, 0, a0:a0 + MLOC, :] = yv[:, :, :S]
        out[:, 1, a0:a0 + MLOC, :] = yv[:, :, S:]
    return out.reshape(B, 2, N, R)
